# revision 36
# baseline (speedup 1.0000x reference)
"""Distributed Trainium2 kernel for nn_DTransformer_35527969473068.

Architecture (from the reference):
  4-layer dense transformer, H=16 heads, D=1024, d_attn=1024 (per head!),
  DV=64, DM=4096, LMAX=1024, V=32000, fp32.

Key structural exploits:
  * The reference reproduces MHAttention's OVERLAPPING slice writes --
    head h writes y[:, h:h+64], later heads overwrite earlier ones.  Net:
    y[:, c] = o[c][:, 0] for c in [0,15), y[:, 15:79] = o[15], rest 0.
  * Wk is folded away on the host: S = q k^T = xn (Wq Wk^T) xn^T, so with
    M = Wq Wk^T precomputed per head (fp32 on host, fp8 on device), the
    kernel computes t = M^T xn once per head and uses xn itself as keys.
  * Final softmax normalization happens on the host: the device returns
    exp(logits) (bf16) plus per-core row sums; no rs-AllReduce needed.

Pipelining: every per-layer stage is split into token halves T0/T1
(tokens 0:512 / 512:1024).  Causality means queries in T0 only attend
keys in T0, so attention/Wo/LN2/MLP for T0 overlap the collectives and
compute of T1.  The y-AllReduce and mlp-AllReduce are per-half, hiding
collective latency behind the other half's compute and keeping the PE
array warm (HAM K=8/8).

Sharding: tensor-parallel over heads (2 heads/core), d_mlp (512/core)
and vocab (4000/core).  Residual stream, LN stats and LN apply in bf16
(fp32 PSUM accumulation), matmuls fp8 DoubleRow / bf16.  All weights
host-pre-arranged into SBUF tile layouts so DMA is contiguous.
"""

import os
import sys

import numpy as np

sys.path.insert(0, "/opt/trn_rl_repo")

L_LAYERS, H, D, DV, DM, LMAX, V = 4, 16, 1024, 64, 4096, 1024, 32000
NCORES = 8
P = 128
NK = D // P            # 8 e-chunks
NI2 = LMAX // 512      # 2 token halves
NJB = LMAX // P        # 8 j-chunks
YW = 80                # padded y width (79 live cols + 1 zero)
YONE = 96              # first ones-column (32-aligned)
YA = 128               # v-hat width: 80 live + 16 zero + 32 ones cols
DMS = DM // NCORES     # 512 d_mlp shard
NUB = DMS // P         # 4 u-chunks
VS = V // NCORES       # 4000 vocab shard
VB = 500               # vocab tile width (8 per core)
NVB = VS // VB

XS = 256.0             # fp8 scale for activations (xn; e4m3 max 240)
WS = 1024.0            # fp8 scale for weights
MQS = 2048.0           # fp8 scale for M = Wq Wk^T (max|M| ~0.095)
QS = 4096.0            # fp8 scale for t = M^T xn
PS = XS * WS           # psum scale after fp8 matmul
PMS = XS * MQS         # psum scale after xn @ M matmul
YS = 4096.0            # fp8 scale for y-AR payload
MS = 4096.0            # fp8 scale for mlp-partial AR payload

N_LAYERS_BUILD = int(os.environ.get("N_LAYERS_BUILD", str(L_LAYERS)))
DEBUG_TAPS = bool(int(os.environ.get("KERNEL_DEBUG_TAPS", "0")))


def build_graph(n_layers=N_LAYERS_BUILD, taps=DEBUG_TAPS):
    from concourse import bacc
    import concourse.bass as bass
    import concourse.mybir as mybir
    import concourse.tile as tile
    from concourse.alu_op_type import AluOpType

    f32 = mybir.dt.float32
    bf16 = mybir.dt.bfloat16
    fp8 = mybir.dt.float8e4
    DR = mybir.MatmulPerfMode.DoubleRow
    AF = mybir.ActivationFunctionType
    ts = bass.ts

    nc = bacc.Bacc("TRN2", target_bir_lowering=False, debug=False,
                   num_devices=NCORES)

    # ---------------- parameters (host pre-arranged tile layouts) -------
    x0_e = nc.declare_dram_parameter("x0", [P, NK, LMAX], bf16, False)
    wm_e, wv_e, wo_e, w1_e, w2_e, ln_e = [], [], [], [], [], []
    for l in range(n_layers):
        wm_e.append(nc.declare_dram_parameter(f"wm{l}", [2, P, NK, D], fp8, False))
        wv_e.append(nc.declare_dram_parameter(f"wv{l}", [2, P, NK, YA], fp8, False))
        wo_e.append(nc.declare_dram_parameter(f"wo{l}", [YW, D], bf16, False))
        w1_e.append(nc.declare_dram_parameter(f"w1{l}", [P, NK, DMS], fp8, False))
        w2_e.append(nc.declare_dram_parameter(f"w2{l}", [P, NUB, D], bf16, False))
        ln_e.append(nc.declare_dram_parameter(f"ln{l}", [P, 4, NK], f32, False))
    lnf_e = nc.declare_dram_parameter("lnf", [P, 2, NK], f32, False)
    wu_e = nc.declare_dram_parameter("wu", [P, NK, VS], fp8, False)
    tri_e = nc.declare_dram_parameter("trimask", [P, P], bf16, False)
    out_e = nc.declare_dram_parameter("out", [LMAX, VS], bf16, True)
    rs_e = nc.declare_dram_parameter("rs", [P, NJB], f32, True)
    taps_e = {}
    if taps:
        for l in range(n_layers):
            taps_e[f"dbg_x{l}"] = nc.declare_dram_parameter(
                f"dbg_x{l}", [P, NK, LMAX], bf16, True)
            taps_e[f"dbg_y{l}"] = nc.declare_dram_parameter(
                f"dbg_y{l}", [YW, LMAX], fp8, True)

    RG = [list(range(NCORES))]

    from contextlib import ExitStack

    with tile.TileContext(nc) as tc:
        with ExitStack() as stack:
            pool = lambda **kw: stack.enter_context(tc.tile_pool(**kw))
            persist = pool(name="persist", bufs=1)
            dram = pool(name="dram", bufs=1, space="DRAM")
            lnw_p = pool(name="lnw", bufs=4)
            # persistent tiles
            xT = persist.tile([P, NK, LMAX], bf16, name="xT")
            xnT = persist.tile([P, NK, LMAX], fp8, name="xnT")
            xn2T = persist.tile([P, NK, LMAX], fp8, name="xn2T")
            ones_bf = persist.tile([P, P], bf16, name="ones_bf")
            ones_f8 = persist.tile([P, P], fp8, name="ones_f8")
            trim = persist.tile([P, P], bf16, name="trim")
            nc.vector.memset(ones_bf[:], 1.0)
            nc.vector.memset(ones_f8[:], 1.0)
            nc.sync.dma_start(trim[:], tri_e[:])
            nc.sync.dma_start(xT[:], x0_e[:])

            def make_layernorm(psS, ptmp, pmv):
                def layernorm_half(g_col, b_col, out_tile, lnp, pref, i2):
                    """xn[:, :, i2-half] = (x-mean)/sd * g + b over the
                    feature axis for tokens in half i2; bf16 stats, fp8 out."""
                    sl = slice(i2 * 512, i2 * 512 + 512)
                    sums = psS.tile([P, 512], f32, name=f"{pref}su", tag="st")
                    sqs = psS.tile([P, 512], f32, name=f"{pref}sq", tag="st")
                    for k in range(NK):
                        sq = ptmp.tile([P, 512], bf16, name=f"{pref}sqt", tag="t")
                        nc.vector.tensor_mul(sq[:], xT[:, k, sl], xT[:, k, sl])
                        nc.tensor.matmul(sums[:], ones_bf[:], xT[:, k, sl],
                                         start=(k == 0), stop=(k == NK - 1))
                        nc.tensor.matmul(sqs[:], ones_bf[:], sq[:],
                                         start=(k == 0), stop=(k == NK - 1))
                    # A' = 1/(D*sd) via rsqrt(D^2 var); D is folded into the
                    # host-staged LN gains: xn = (x*A' + B')*(g*XS*D) + b*XS
                    m2 = pmv.tile([P, 512], f32, name=f"{pref}m2", tag="m")
                    v_sb = pmv.tile([P, 512], f32, name=f"{pref}v", tag="v")
                    Af = pmv.tile([P, 512], f32, name=f"{pref}Af", tag="Af")
                    Ab = pmv.tile([P, 512], bf16, name=f"{pref}A", tag="A")
                    Bb = pmv.tile([P, 512], bf16, name=f"{pref}B", tag="B")
                    nc.scalar.activation(m2[:], sums[:], AF.Square)
                    nc.vector.scalar_tensor_tensor(
                        v_sb[:], sqs[:], float(D), m2[:],
                        AluOpType.mult, AluOpType.subtract)
                    nc.scalar.activation(Ab[:], v_sb[:],
                                         AF.Abs_reciprocal_sqrt)
                    nc.vector.scalar_tensor_tensor(
                        Bb[:], sums[:], -1.0 / D, Ab[:],
                        AluOpType.mult, AluOpType.mult)
                    for k in range(NK):
                        t = ptmp.tile([P, 512], bf16, name=f"{pref}at", tag="t")
                        nc.vector.tensor_mul(t[:], xT[:, k, sl], Ab[:])
                        nc.vector.tensor_add(t[:], t[:], Bb[:])
                        nc.vector.tensor_scalar(
                            out_tile[:, k, sl], t[:],
                            lnp[:, g_col:g_col + 1, k],
                            lnp[:, b_col:b_col + 1, k],
                            AluOpType.mult, AluOpType.add)
                return layernorm_half

            # ---------------- layers ----------------
            wm_p = pool(name="wm", bufs=2)
            qk_p = pool(name="qk", bufs=2)
            es_p = pool(name="es", bufs=3)
            vv_p = pool(name="vv", bufs=2)
            ya_p = pool(name="ya", bufs=2)
            lnp_p = pool(name="lnparam", bufs=2)
            w12_p = pool(name="w12", bufs=1)
            gel_p = pool(name="gel", bufs=1)
            mst_p = pool(name="mstage", bufs=3)
            wu_p = pool(name="wu", bufs=1)
            ev_p = pool(name="ev", bufs=2)
            fin_p = pool(name="fin", bufs=1)
            psK = pool(name="psK", bufs=4, space="PSUM")
            psU = pool(name="psU", bufs=2, space="PSUM")
            psS = pool(name="psS", bufs=2, space="PSUM")
            lntmp_p = pool(name="lntmp", bufs=3)
            lnmv_p = pool(name="lnmv", bufs=2)
            if True:
                lnfp = persist.tile([P, 2, NK], f32, name="lnfp")
                nc.sync.dma_start(lnfp[:], lnf_e[:])
                acc = fin_p.tile([P, NJB * NVB], f32, name="acc")
                rs = fin_p.tile([P, NJB], f32, name="rs")
                wuf = wu_p.tile([P, NK, VS], fp8, name="wuf")

                layernorm_half = make_layernorm(psS, lntmp_p, lnmv_p)

                def unembed_half(ih):
                    layernorm_half(0, 1, xnT, lnfp, f"lnfh{ih}", ih)
                    if True:
                        psl = psK
                        for ib in range(ih * 4, ih * 4 + 4):
                            expV = ev_p.tile([P, VS], bf16, name=f"expV{ib}",
                                             tag="ev")
                            for vg in range(NVB):
                                pl = psl.tile([P, VB], f32, name="pl", tag="p")
                                for kg in range(NK // 2):
                                    nc.tensor.matmul(
                                        pl[:],
                                        xnT[:, 2 * kg:2 * kg + 2, ts(ib, P)],
                                        wuf[:, 2 * kg:2 * kg + 2, ts(vg, VB)],
                                        start=(kg == 0),
                                        stop=(kg == NK // 2 - 1),
                                        perf_mode=DR)
                                nc.scalar.activation(
                                    expV[:, ts(vg, VB)], pl[:], AF.Exp,
                                    scale=1.0 / PS,
                                    accum_out=acc[:, ib * NVB + vg:
                                                  ib * NVB + vg + 1])
                            nc.vector.reduce_sum(rs[:, ib:ib + 1],
                                                 acc[:, ts(ib, NVB)],
                                                 mybir.AxisListType.X)
                            nc.sync.dma_start(out_e[ts(ib, P), :], expV[:])
                def attn_half(l, hi, ih, wm, wv, tT, vh, yT):
                    """Attention for head-pair hi, query-token half ih."""
                    lo, hi2 = ih * 512, ih * 512 + 512
                    # t = M^T xn for this half's query columns
                    for db in range(NK):
                        pp = psK.tile([P, 512], f32, name="pq", tag="p")
                        for kg in range(NK // 2):
                            nc.tensor.matmul(
                                pp[:],
                                wm[:, 2 * kg:2 * kg + 2, ts(db, P)],
                                xnT[:, 2 * kg:2 * kg + 2, lo:hi2],
                                start=(kg == 0),
                                stop=(kg == NK // 2 - 1),
                                perf_mode=DR)
                        if db % 2 == 0:
                            nc.scalar.mul(
                                tT[:, db, lo:hi2], pp[:], QS / PMS)
                        else:
                            nc.vector.tensor_scalar_mul(
                                tT[:, db, lo:hi2], pp[:], QS / PMS)
                    # v-hat for this half's key blocks
                    for jb in range(ih * 4, ih * 4 + 4):
                        pv = psK.tile([P, YA], f32, name="pv", tag="p")
                        for k in range(NK):
                            nc.tensor.matmul(
                                pv[:], xnT[:, k, ts(jb, P)], wv[:, k, :],
                                start=(k == 0), stop=(k == NK - 1))
                        nc.scalar.mul(vh[:, jb, :], pv[:], 1.0 / PS)
                        nc.vector.memset(vh[:, jb, YONE:YA], 1.0)
                    # scores^T -> exp -> U accumulation over key blocks
                    if True:
                        last = ih * 4 + 3
                        pu = psU.tile([YA, 512], f32, name="pu", tag="u")
                        for jb in range(last + 1):
                            jlo = jb * P
                            ex = es_p.tile([P, 512], bf16,
                                           name=f"ex{l}{hi}{ih}{jb}", tag="ex")
                            ps = psK.tile([P, 512], f32, name="ps", tag="p")
                            for kg in range(NK // 2):
                                nc.tensor.matmul(
                                    ps[:], xnT[:, 2 * kg:2 * kg + 2, ts(jb, P)],
                                    tT[:, 2 * kg:2 * kg + 2, lo:hi2],
                                    start=(kg == 0),
                                    stop=(kg == NK // 2 - 1),
                                    perf_mode=DR)
                            vs = max(lo, jlo)
                            if vs > lo:
                                nc.vector.memset(ex[:, 0:vs - lo], 0.0)
                            nc.scalar.activation(
                                ex[:, vs - lo:512], ps[:, vs - lo:512],
                                AF.Exp, scale=1.0 / (32.0 * QS * XS))
                            if lo <= jlo < hi2:
                                nc.vector.tensor_mul(
                                    ex[:, jlo - lo:jlo - lo + P],
                                    ex[:, jlo - lo:jlo - lo + P], trim[:])
                            nc.tensor.matmul(
                                pu[:], vh[:, jb, :], ex[:],
                                start=(jb == 0), stop=(jb == last))
                        # normalize and accumulate into yT
                        with tc.tile_pool(name=f"nrm{l}{hi}{ih}", bufs=2) as nrm_p:
                            dn = nrm_p.tile([32, 512], f32, name="dn", tag="dn")
                            nc.scalar.copy(dn[:], pu[YONE:YA, :])
                            rb = nrm_p.tile([32, 512], f32, name="rb", tag="rb")
                            nc.vector.reciprocal_approx_fast(rb[:], dn[:])
                            u2f = (None if hi == 0 else
                                   nrm_p.tile([YW, 512], fp8, name="u2", tag="u2"))
                            for c0, cw in ((0, 32), (32, 32), (64, 16)):
                                if hi == 0:
                                    nc.vector.scalar_tensor_tensor(
                                        yT[c0:c0 + cw, lo:hi2],
                                        pu[c0:c0 + cw, :], YS, rb[0:cw, :],
                                        AluOpType.mult, AluOpType.mult)
                                else:
                                    nc.vector.scalar_tensor_tensor(
                                        u2f[c0:c0 + cw, :],
                                        pu[c0:c0 + cw, :], YS, rb[0:cw, :],
                                        AluOpType.mult, AluOpType.mult)
                                    nc.vector.tensor_add(
                                        yT[c0:c0 + cw, lo:hi2],
                                        yT[c0:c0 + cw, lo:hi2],
                                        u2f[c0:c0 + cw, :])

                def mlp_half(l, ih, lnp, wo, w1, w2, y_out, m_in, m_out,
                             land=None):
                    """Wo + LN2 + MLP + m-AR for token half ih (after y-AR)."""
                    lo, hi2 = ih * 512, ih * 512 + 512
                    yb8 = ya_p.tile([YW, 512], fp8, name=f"yb8{l}{ih}", tag="yb8")
                    nc.sync.dma_start(yb8[:], y_out[:])
                    ybb = ya_p.tile([YW, 512], bf16, name=f"ybb{l}{ih}", tag="ybb")
                    nc.scalar.mul(ybb[:], yb8[:], 1.0 / YS)
                    for k in range(NK):
                        po = psK.tile([P, 512], f32, name="po", tag="p")
                        nc.tensor.matmul(po[:], wo[:, ts(k, P)], ybb[:],
                                         start=True, stop=True)
                        nc.vector.tensor_add(
                            xT[:, k, lo:hi2], xT[:, k, lo:hi2], po[:])
                    layernorm_half(2, 3, xn2T, lnp, f"l{l}n2h{ih}", ih)
                    if land is not None:
                        mlp_land(*land)
                    gl = gel_p.tile([P, NUB, 512], bf16, name=f"gl{l}{ih}",
                                    tag="gl")
                    for ub in range(NUB):
                        pm = psK.tile([P, 512], f32, name="pm", tag="p")
                        for kg in range(NK // 2):
                            nc.tensor.matmul(
                                pm[:], w1[:, 2 * kg:2 * kg + 2, ts(ub, P)],
                                xn2T[:, 2 * kg:2 * kg + 2, lo:hi2],
                                start=(kg == 0),
                                stop=(kg == NK // 2 - 1),
                                perf_mode=DR)
                        nc.scalar.activation(
                            gl[:, ub, :], pm[:],
                            AF.Gelu_apprx_tanh, scale=1.0 / PS)
                    # x += xn2/XS while W2 runs
                    for k in range(NK):
                        nc.vector.scalar_tensor_tensor(
                            xT[:, k, lo:hi2], xn2T[:, k, lo:hi2], 1.0 / XS,
                            xT[:, k, lo:hi2], AluOpType.mult, AluOpType.add)
                    for k in range(NK):
                        mc = mst_p.tile([P, 512], fp8, name="mc", tag="mc")
                        pp = psK.tile([P, 512], f32, name="pp", tag="p")
                        for ub in range(NUB):
                            nc.tensor.matmul(
                                pp[:], w2[:, ub, ts(k, P)], gl[:, ub, :],
                                start=(ub == 0), stop=(ub == NUB - 1))
                        nc.scalar.mul(mc[:], pp[:], MS)
                        nc.sync.dma_start(m_in[:, k, :], mc[:])
                    nc.gpsimd.collective_compute(
                        "AllReduce", AluOpType.add, replica_groups=RG,
                        ins=[m_in.opt()], outs=[m_out.opt()])

                def mlp_land(l, ih, m_out):
                    """x += mlp result for half ih (after its m-AR)."""
                    lo, hi2 = ih * 512, ih * 512 + 512
                    for k in range(NK):
                        mr = mst_p.tile([P, 512], fp8, name="mr", tag="mr")
                        nc.sync.dma_start(mr[:], m_out[:, k, :])
                        nc.vector.scalar_tensor_tensor(
                            xT[:, k, lo:hi2], mr[:], 1.0 / MS,
                            xT[:, k, lo:hi2], AluOpType.mult, AluOpType.add)

                pending_land = None
                for l in range(n_layers):
                    lnp = lnp_p.tile([P, 4, NK], f32, name=f"lnp{l}", tag="lnp")
                    nc.sync.dma_start(lnp[:], ln_e[l][:])
                    wo = lnw_p.tile([YW, D], bf16, name=f"wo{l}", tag="wo")
                    nc.sync.dma_start(wo[:], wo_e[l][:])
                    w1 = w12_p.tile([P, NK, DMS], fp8, name=f"w1{l}", tag="w1")
                    w2 = w12_p.tile([P, NUB, D], bf16, name=f"w2{l}", tag="w2")
                    nc.sync.dma_start(w1[:], w1_e[l][:])
                    nc.sync.dma_start(w2[:], w2_e[l][:])
                    wms, wvs, tTs, vhs = [], [], [], []
                    for hi in range(2):
                        wm = wm_p.tile([P, NK, D], fp8, name=f"wm{l}{hi}", tag="w")
                        nc.sync.dma_start(wm[:], wm_e[l][hi])
                        wv = vv_p.tile([P, NK, YA], fp8, name=f"wv{l}{hi}",
                                       tag="wv")
                        nc.sync.dma_start(wv[:], wv_e[l][hi])
                        wms.append(wm)
                        wvs.append(wv)
                        tTs.append(qk_p.tile([P, NK, LMAX], fp8,
                                             name=f"tT{l}{hi}", tag="qk"))
                        vhs.append(vv_p.tile([P, NJB, YA], bf16,
                                             name=f"vh{l}{hi}", tag="vh"))
                    yT = ya_p.tile([YW, LMAX], fp8, name=f"yT{l}", tag="yT")
                    y_in = [dram.tile([YW, 512], fp8, name=f"yin{l}{ih}",
                                      tag=f"yin{ih}", bufs=2) for ih in range(2)]
                    y_out = [dram.tile([YW, 512], fp8, name=f"yout{l}{ih}",
                                       tag=f"yout{ih}", addr_space="Shared",
                                       bufs=2) for ih in range(2)]
                    m_in = [dram.tile([P, NK, 512], fp8, name=f"min{l}{ih}",
                                      tag=f"min{ih}", bufs=2) for ih in range(2)]
                    m_out = [dram.tile([P, NK, 512], fp8, name=f"mout{l}{ih}",
                                       tag=f"mout{ih}", addr_space="Shared",
                                       bufs=2) for ih in range(2)]

                    layernorm_half(0, 1, xnT, lnp, f"l{l}n1h0", 0)
                    attn_half(l, 0, 0, wms[0], wvs[0], tTs[0], vhs[0], yT)
                    if pending_land is not None:
                        mlp_land(*pending_land)
                        pending_land = None
                    layernorm_half(0, 1, xnT, lnp, f"l{l}n1h1", 1)
                    attn_half(l, 1, 0, wms[1], wvs[1], tTs[1], vhs[1], yT)
                    nc.sync.dma_start(y_in[0][:], yT[:, 0:512])
                    nc.gpsimd.collective_compute(
                        "AllReduce", AluOpType.add, replica_groups=RG,
                        ins=[y_in[0].opt()], outs=[y_out[0].opt()])
                    for hi in range(2):
                        attn_half(l, hi, 1, wms[hi], wvs[hi],
                                  tTs[hi], vhs[hi], yT)
                    nc.sync.dma_start(y_in[1][:], yT[:, 512:1024])
                    nc.gpsimd.collective_compute(
                        "AllReduce", AluOpType.add, replica_groups=RG,
                        ins=[y_in[1].opt()], outs=[y_out[1].opt()])
                    if l == n_layers - 1:
                        for kg in range(NK // 2):
                            nc.sync.dma_start(wuf[:, 2 * kg:2 * kg + 2, :],
                                              wu_e[:, 2 * kg:2 * kg + 2, :])
                    mlp_half(l, 0, lnp, wo, w1, w2, y_out[0],
                             m_in[0], m_out[0])
                    mlp_half(l, 1, lnp, wo, w1, w2, y_out[1],
                             m_in[1], m_out[1], land=(l, 0, m_out[0]))
                    pending_land = (l, 1, m_out[1])
                    if taps:
                        mlp_land(*pending_land)
                        pending_land = None
                        nc.sync.dma_start(taps_e[f"dbg_x{l}"][:], xT[:])
                        yta = ya_p.tile([YW, LMAX], fp8, name=f"yta{l}",
                                        tag="yta")
                        nc.sync.dma_start(yta[:, 0:512], y_out[0][:])
                        nc.sync.dma_start(yta[:, 512:1024], y_out[1][:])
                        nc.sync.dma_start(taps_e[f"dbg_y{l}"][:], yta[:])

                # ------- final LN + unembed exp (host normalizes), -------
                # interleaved with the last layer's second m-AR landing
                unembed_half(0)
                if pending_land is not None:
                    mlp_land(*pending_land)
                unembed_half(1)
                nc.sync.dma_start(rs_e[:], rs[:])

    nc.compile()
    return nc


def shard_inputs(inputs, n_layers=N_LAYERS_BUILD):
    import ml_dtypes
    bf = ml_dtypes.bfloat16
    f8 = ml_dtypes.float8_e4m3

    x_ids = np.asarray(inputs["x_ids"]).astype(np.int64)
    we = np.asarray(inputs["word_emb"], np.float32)
    pe = np.asarray(inputs["pos_emb"], np.float32)
    x0t = np.ascontiguousarray((we[x_ids] + pe).T)  # (D, LMAX) f32
    # tile layout [p, k, i]: feature e = k*128 + p
    x0r = np.ascontiguousarray(
        x0t.reshape(NK, P, LMAX).transpose(1, 0, 2)).astype(bf)

    Wq = np.asarray(inputs["Wq"], np.float32)
    Wk = np.asarray(inputs["Wk"], np.float32)
    Wv = np.asarray(inputs["Wv"], np.float32)
    Wo = np.asarray(inputs["Wo"], np.float32)
    W1 = np.asarray(inputs["W1"], np.float32)
    W2 = np.asarray(inputs["W2"], np.float32)
    g1, b1 = np.asarray(inputs["g1"], np.float32), np.asarray(inputs["b1"], np.float32)
    g2, b2 = np.asarray(inputs["g2"], np.float32), np.asarray(inputs["b2"], np.float32)
    gf, bfv = np.asarray(inputs["gf"], np.float32), np.asarray(inputs["bf"], np.float32)
    Wu = np.asarray(inputs["Wu"], np.float32)

    tri = np.triu(np.ones((P, P), np.float32)).astype(bf)  # valid j'<=i'

    def feat_major(a):
        # (D, cols) -> (P, NK, cols) with feature e = k*128 + p
        return np.ascontiguousarray(
            a.reshape(NK, P, -1).transpose(1, 0, 2))

    # M = Wq Wk^T per (layer, head): [l, h, d, f]
    M_all = np.matmul(Wq[:n_layers], Wk[:n_layers].transpose(0, 1, 3, 2))

    in_maps = []
    for c in range(NCORES):
        m = {"x0": x0r, "trimask": tri,
             "lnf": np.ascontiguousarray(
                 (np.stack([gf * D, bfv]) * XS).astype(np.float32)
                 .reshape(2, NK, P).transpose(2, 0, 1)),
             "wu": (feat_major(Wu[:, c * VS:(c + 1) * VS]) * WS).astype(f8)}
        for l in range(n_layers):
            h0 = 2 * c
            m[f"wm{l}"] = np.stack([
                (feat_major(M_all[l, h0 + hi]) * MQS).astype(f8)
                for hi in range(2)])
            wv_eff = np.zeros((2, D, YA), np.float32)
            for hi in range(2):
                h = h0 + hi
                if h < 15:
                    wv_eff[hi, :, h] = Wv[l, h, :, 0]
                else:
                    wv_eff[hi, :, 15:15 + DV] = Wv[l, h]
                # cols 79..95 stay zero; col 96 becomes the ones column
                # (set on-chip after the matmul)
            m[f"wv{l}"] = np.stack([
                (feat_major(wv_eff[hi]) * WS).astype(f8) for hi in range(2)])
            wo80 = np.zeros((YW, D), np.float32)
            wo80[:79] = Wo[l][:79]
            m[f"wo{l}"] = wo80.astype(bf)
            m[f"w1{l}"] = (feat_major(
                W1[l][:, c * DMS:(c + 1) * DMS]) * WS).astype(f8)
            # w2 layout [p, u, d]: dm row = u*128 + p within this core's shard
            m[f"w2{l}"] = np.ascontiguousarray(
                W2[l][c * DMS:(c + 1) * DMS].reshape(NUB, P, D)
                .transpose(1, 0, 2)).astype(bf)
            m[f"ln{l}"] = np.ascontiguousarray(
                (np.stack([g1[l] * D, b1[l], g2[l] * D, b2[l]]) * XS)
                .astype(np.float32).reshape(4, NK, P).transpose(2, 0, 1))
        in_maps.append(m)
    return in_maps


_GRAPH_CACHE = {}


def _ensure_ntff_hook():
    """The agent image's antenv lacks axon_hooks; recreate it so
    run_bass_kernel_spmd(trace=True) can capture NTFF profiles."""
    import types
    try:
        import antenv.axon_hooks  # noqa: F401
        return
    except ImportError:
        pass
    import importlib.util
    import antenv
    spec = importlib.util.spec_from_file_location(
        "_trn_boot_for_hook", "/root/.axon_site/trn_agent_boot/trn_boot.py")
    tb = importlib.util.module_from_spec(spec)
    spec.loader.exec_module(tb)
    mod = types.ModuleType("antenv.axon_hooks")
    hook_box = [tb._ntff_profile_via_ctypes("/opt/axon/libaxon_pjrt.so")]
    mod.set_axon_ntff_profile_hook = lambda h: hook_box.__setitem__(0, h)
    mod.get_axon_ntff_profile_hook = lambda: hook_box[0]
    sys.modules["antenv.axon_hooks"] = mod
    antenv.axon_hooks = mod


def run(inputs, trace=False, n_layers=N_LAYERS_BUILD):
    from concourse.bass_utils import run_bass_kernel_spmd
    if trace:
        _ensure_ntff_hook()
    key = (n_layers, DEBUG_TAPS)
    if key not in _GRAPH_CACHE:
        _GRAPH_CACHE[key] = build_graph(n_layers)
    nc = _GRAPH_CACHE[key]
    in_maps = shard_inputs(inputs, n_layers)
    res = run_bass_kernel_spmd(nc, in_maps, list(range(NCORES)), trace=trace)
    expv = np.concatenate(
        [np.asarray(res.results[c]["out"], np.float32) for c in range(NCORES)],
        axis=1)                                        # (LMAX, V)
    # denominators: rs[p, ib] is the row sum of token ib*128+p on each core
    denom = np.zeros(LMAX, np.float64)
    for c in range(NCORES):
        rs = np.asarray(res.results[c]["rs"], np.float64)  # (P, NJB)
        denom += rs.T.reshape(LMAX)
    out = (expv / denom[:, None]).astype(np.float32)
    return out, res


def kernel(**inputs):
    out, _ = run(inputs)
    return out


# revision 37
# speedup vs baseline: 1.0272x; 1.0272x over previous
"""Distributed Trainium2 kernel for nn_DTransformer_35527969473068.

Architecture (from the reference):
  4-layer dense transformer, H=16 heads, D=1024, d_attn=1024 (per head!),
  DV=64, DM=4096, LMAX=1024, V=32000, fp32.

Key structural exploits:
  * The reference reproduces MHAttention's OVERLAPPING slice writes --
    head h writes y[:, h:h+64], later heads overwrite earlier ones.  Net:
    y[:, c] = o[c][:, 0] for c in [0,15), y[:, 15:79] = o[15], rest 0.
  * Wk is folded away on the host: S = q k^T = xn (Wq Wk^T) xn^T, so with
    M = Wq Wk^T precomputed per head (fp32 on host, fp8 on device), the
    kernel computes t = M^T xn once per head and uses xn itself as keys.
  * Final softmax normalization happens on the host: the device returns
    exp(logits) (bf16) plus per-core row sums; no rs-AllReduce needed.

Pipelining: every per-layer stage is split into token halves T0/T1
(tokens 0:512 / 512:1024).  Causality means queries in T0 only attend
keys in T0, so attention/Wo/LN2/MLP for T0 overlap the collectives and
compute of T1.  The y-AllReduce and mlp-AllReduce are per-half, hiding
collective latency behind the other half's compute and keeping the PE
array warm (HAM K=8/8).

Sharding: tensor-parallel over heads (2 heads/core), d_mlp (512/core)
and vocab (4000/core).  Residual stream, LN stats and LN apply in bf16
(fp32 PSUM accumulation), matmuls fp8 DoubleRow / bf16.  All weights
host-pre-arranged into SBUF tile layouts so DMA is contiguous.
"""

import os
import sys

import numpy as np

sys.path.insert(0, "/opt/trn_rl_repo")

L_LAYERS, H, D, DV, DM, LMAX, V = 4, 16, 1024, 64, 4096, 1024, 32000
NCORES = 8
P = 128
NK = D // P            # 8 e-chunks
NI2 = LMAX // 512      # 2 token halves
NJB = LMAX // P        # 8 j-chunks
YW = 80                # padded y width (79 live cols + 1 zero)
YONE = 96              # first ones-column (32-aligned)
YA = 128               # v-hat width: 80 live + 16 zero + 32 ones cols
DMS = DM // NCORES     # 512 d_mlp shard
NUB = DMS // P         # 4 u-chunks
VS = V // NCORES       # 4000 vocab shard
VB = 500               # vocab tile width (8 per core)
NVB = VS // VB

XS = 256.0             # fp8 scale for activations (xn; e4m3 max 240)
WS = 1024.0            # fp8 scale for weights
MQS = 2048.0           # fp8 scale for M = Wq Wk^T (max|M| ~0.095)
QS = 4096.0            # fp8 scale for t = M^T xn
PS = XS * WS           # psum scale after fp8 matmul
PMS = XS * MQS         # psum scale after xn @ M matmul
YS = 4096.0            # fp8 scale for y-AR payload
MS = 4096.0            # fp8 scale for mlp-partial AR payload

N_LAYERS_BUILD = int(os.environ.get("N_LAYERS_BUILD", str(L_LAYERS)))
DEBUG_TAPS = bool(int(os.environ.get("KERNEL_DEBUG_TAPS", "0")))


def build_graph(n_layers=N_LAYERS_BUILD, taps=DEBUG_TAPS):
    from concourse import bacc
    import concourse.bass as bass
    import concourse.mybir as mybir
    import concourse.tile as tile
    from concourse.alu_op_type import AluOpType

    f32 = mybir.dt.float32
    bf16 = mybir.dt.bfloat16
    fp8 = mybir.dt.float8e4
    DR = mybir.MatmulPerfMode.DoubleRow
    AF = mybir.ActivationFunctionType
    ts = bass.ts

    nc = bacc.Bacc("TRN2", target_bir_lowering=False, debug=False,
                   num_devices=NCORES)

    # ---------------- parameters (host pre-arranged tile layouts) -------
    x0_e = nc.declare_dram_parameter("x0", [P, NK, LMAX], bf16, False)
    wm_e, wv_e, wo_e, w1_e, w2_e, ln_e = [], [], [], [], [], []
    for l in range(n_layers):
        wm_e.append(nc.declare_dram_parameter(f"wm{l}", [2, P, NK, D], fp8, False))
        wv_e.append(nc.declare_dram_parameter(f"wv{l}", [2, P, NK, YA], fp8, False))
        wo_e.append(nc.declare_dram_parameter(f"wo{l}", [YW, D], bf16, False))
        w1_e.append(nc.declare_dram_parameter(f"w1{l}", [P, NK, DMS], fp8, False))
        w2_e.append(nc.declare_dram_parameter(f"w2{l}", [P, NUB, D], bf16, False))
        ln_e.append(nc.declare_dram_parameter(f"ln{l}", [P, 4, NK], f32, False))
    lnf_e = nc.declare_dram_parameter("lnf", [P, 2, NK], f32, False)
    wu_e = nc.declare_dram_parameter("wu", [P, NK, VS], fp8, False)
    tri_e = nc.declare_dram_parameter("trimask", [P, P], bf16, False)
    out_e = nc.declare_dram_parameter("out", [LMAX, VS], bf16, True)
    rs_e = nc.declare_dram_parameter("rs", [P, NJB], f32, True)
    taps_e = {}
    if taps:
        for l in range(n_layers):
            taps_e[f"dbg_x{l}"] = nc.declare_dram_parameter(
                f"dbg_x{l}", [P, NK, LMAX], bf16, True)
            taps_e[f"dbg_y{l}"] = nc.declare_dram_parameter(
                f"dbg_y{l}", [YW, LMAX], fp8, True)

    RG = [list(range(NCORES))]

    from contextlib import ExitStack

    with tile.TileContext(nc) as tc:
        with ExitStack() as stack:
            pool = lambda **kw: stack.enter_context(tc.tile_pool(**kw))
            persist = pool(name="persist", bufs=1)
            dram = pool(name="dram", bufs=1, space="DRAM")
            lnw_p = pool(name="lnw", bufs=4)
            # persistent tiles
            xT = persist.tile([P, NK, LMAX], bf16, name="xT")
            xnT = persist.tile([P, NK, LMAX], fp8, name="xnT")
            xn2T = persist.tile([P, NK, LMAX], fp8, name="xn2T")
            ones_bf = persist.tile([P, P], bf16, name="ones_bf")
            ones_f8 = persist.tile([P, P], fp8, name="ones_f8")
            trim = persist.tile([P, P], bf16, name="trim")
            nc.vector.memset(ones_bf[:], 1.0)
            nc.vector.memset(ones_f8[:], 1.0)
            nc.sync.dma_start(trim[:], tri_e[:])
            nc.sync.dma_start(xT[:], x0_e[:])

            def make_layernorm(psS, ptmp, pmv):
                def layernorm_half(g_col, b_col, out_tile, lnp, pref, i2):
                    """xn[:, :, i2-half] = (x-mean)/sd * g + b over the
                    feature axis for tokens in half i2; bf16 stats, fp8 out."""
                    sl = slice(i2 * 512, i2 * 512 + 512)
                    sums = psS.tile([P, 512], f32, name=f"{pref}su", tag="st")
                    sqs = psS.tile([P, 512], f32, name=f"{pref}sq", tag="st")
                    for k in range(NK):
                        sq = ptmp.tile([P, 512], bf16, name=f"{pref}sqt", tag="t")
                        nc.vector.tensor_mul(sq[:], xT[:, k, sl], xT[:, k, sl])
                        nc.tensor.matmul(sums[:], ones_bf[:], xT[:, k, sl],
                                         start=(k == 0), stop=(k == NK - 1))
                        nc.tensor.matmul(sqs[:], ones_bf[:], sq[:],
                                         start=(k == 0), stop=(k == NK - 1))
                    # A' = 1/(D*sd) via rsqrt(D^2 var); D is folded into the
                    # host-staged LN gains: xn = (x*A' + B')*(g*XS*D) + b*XS
                    m2 = pmv.tile([P, 512], f32, name=f"{pref}m2", tag="m")
                    v_sb = pmv.tile([P, 512], f32, name=f"{pref}v", tag="v")
                    Af = pmv.tile([P, 512], f32, name=f"{pref}Af", tag="Af")
                    Ab = pmv.tile([P, 512], bf16, name=f"{pref}A", tag="A")
                    Bb = pmv.tile([P, 512], bf16, name=f"{pref}B", tag="B")
                    nc.scalar.activation(m2[:], sums[:], AF.Square)
                    nc.vector.scalar_tensor_tensor(
                        v_sb[:], sqs[:], float(D), m2[:],
                        AluOpType.mult, AluOpType.subtract)
                    nc.scalar.activation(Ab[:], v_sb[:],
                                         AF.Abs_reciprocal_sqrt)
                    nc.vector.scalar_tensor_tensor(
                        Bb[:], sums[:], -1.0 / D, Ab[:],
                        AluOpType.mult, AluOpType.mult)
                    for k in range(NK):
                        t = ptmp.tile([P, 512], bf16, name=f"{pref}at", tag="t")
                        nc.vector.tensor_mul(t[:], xT[:, k, sl], Ab[:])
                        nc.vector.tensor_add(t[:], t[:], Bb[:])
                        if k % 2 == 0:
                            nc.vector.tensor_scalar(
                                out_tile[:, k, sl], t[:],
                                lnp[:, g_col:g_col + 1, k],
                                lnp[:, b_col:b_col + 1, k],
                                AluOpType.mult, AluOpType.add)
                        else:
                            nc.scalar.activation(
                                out_tile[:, k, sl], t[:], AF.Identity,
                                bias=lnp[:, b_col:b_col + 1, k],
                                scale=lnp[:, g_col:g_col + 1, k])
                return layernorm_half

            # ---------------- layers ----------------
            wm_p = pool(name="wm", bufs=2)
            qk_p = pool(name="qk", bufs=2)
            es_p = pool(name="es", bufs=3)
            vv_p = pool(name="vv", bufs=2)
            ya_p = pool(name="ya", bufs=2)
            lnp_p = pool(name="lnparam", bufs=2)
            w12_p = pool(name="w12", bufs=1)
            gel_p = pool(name="gel", bufs=1)
            mst_p = pool(name="mstage", bufs=3)
            wu_p = pool(name="wu", bufs=1)
            ev_p = pool(name="ev", bufs=2)
            fin_p = pool(name="fin", bufs=1)
            psK = pool(name="psK", bufs=4, space="PSUM")
            psU = pool(name="psU", bufs=2, space="PSUM")
            psS = pool(name="psS", bufs=2, space="PSUM")
            lntmp_p = pool(name="lntmp", bufs=3)
            lnmv_p = pool(name="lnmv", bufs=2)
            if True:
                lnfp = persist.tile([P, 2, NK], f32, name="lnfp")
                nc.sync.dma_start(lnfp[:], lnf_e[:])
                acc = fin_p.tile([P, NJB * NVB], f32, name="acc")
                rs = fin_p.tile([P, NJB], f32, name="rs")
                wuf = wu_p.tile([P, NK, VS], fp8, name="wuf")

                layernorm_half = make_layernorm(psS, lntmp_p, lnmv_p)

                def unembed_half(ih):
                    layernorm_half(0, 1, xnT, lnfp, f"lnfh{ih}", ih)
                    if True:
                        psl = psK
                        for ib in range(ih * 4, ih * 4 + 4):
                            expV = ev_p.tile([P, VS], bf16, name=f"expV{ib}",
                                             tag="ev")
                            for vg in range(NVB):
                                pl = psl.tile([P, VB], f32, name="pl", tag="p")
                                for kg in range(NK // 2):
                                    nc.tensor.matmul(
                                        pl[:],
                                        xnT[:, 2 * kg:2 * kg + 2, ts(ib, P)],
                                        wuf[:, 2 * kg:2 * kg + 2, ts(vg, VB)],
                                        start=(kg == 0),
                                        stop=(kg == NK // 2 - 1),
                                        perf_mode=DR)
                                nc.scalar.activation(
                                    expV[:, ts(vg, VB)], pl[:], AF.Exp,
                                    scale=1.0 / PS,
                                    accum_out=acc[:, ib * NVB + vg:
                                                  ib * NVB + vg + 1])
                            nc.vector.reduce_sum(rs[:, ib:ib + 1],
                                                 acc[:, ts(ib, NVB)],
                                                 mybir.AxisListType.X)
                            nc.sync.dma_start(out_e[ts(ib, P), :], expV[:])
                def attn_half(l, hi, ih, wm, wv, tT, vh, yT):
                    """Attention for head-pair hi, query-token half ih."""
                    lo, hi2 = ih * 512, ih * 512 + 512
                    # t = M^T xn for this half's query columns
                    for db in range(NK):
                        pp = psK.tile([P, 512], f32, name="pq", tag="p")
                        for kg in range(NK // 2):
                            nc.tensor.matmul(
                                pp[:],
                                wm[:, 2 * kg:2 * kg + 2, ts(db, P)],
                                xnT[:, 2 * kg:2 * kg + 2, lo:hi2],
                                start=(kg == 0),
                                stop=(kg == NK // 2 - 1),
                                perf_mode=DR)
                        if db % 2 == 0:
                            nc.scalar.mul(
                                tT[:, db, lo:hi2], pp[:], QS / PMS)
                        else:
                            nc.vector.tensor_scalar_mul(
                                tT[:, db, lo:hi2], pp[:], QS / PMS)
                    # v-hat for this half's key blocks
                    for jb in range(ih * 4, ih * 4 + 4):
                        pv = psK.tile([P, YA], f32, name="pv", tag="p")
                        for k in range(NK):
                            nc.tensor.matmul(
                                pv[:], xnT[:, k, ts(jb, P)], wv[:, k, :],
                                start=(k == 0), stop=(k == NK - 1))
                        nc.scalar.mul(vh[:, jb, :], pv[:], 1.0 / PS)
                        nc.vector.memset(vh[:, jb, YONE:YA], 1.0)
                    # scores^T -> exp -> U accumulation over key blocks
                    if True:
                        last = ih * 4 + 3
                        pu = psU.tile([YA, 512], f32, name="pu", tag="u")
                        for jb in range(last + 1):
                            jlo = jb * P
                            ex = es_p.tile([P, 512], bf16,
                                           name=f"ex{l}{hi}{ih}{jb}", tag="ex")
                            ps = psK.tile([P, 512], f32, name="ps", tag="p")
                            for kg in range(NK // 2):
                                nc.tensor.matmul(
                                    ps[:], xnT[:, 2 * kg:2 * kg + 2, ts(jb, P)],
                                    tT[:, 2 * kg:2 * kg + 2, lo:hi2],
                                    start=(kg == 0),
                                    stop=(kg == NK // 2 - 1),
                                    perf_mode=DR)
                            vs = max(lo, jlo)
                            if vs > lo:
                                nc.vector.memset(ex[:, 0:vs - lo], 0.0)
                            nc.scalar.activation(
                                ex[:, vs - lo:512], ps[:, vs - lo:512],
                                AF.Exp, scale=1.0 / (32.0 * QS * XS))
                            if lo <= jlo < hi2:
                                nc.vector.tensor_mul(
                                    ex[:, jlo - lo:jlo - lo + P],
                                    ex[:, jlo - lo:jlo - lo + P], trim[:])
                            nc.tensor.matmul(
                                pu[:], vh[:, jb, :], ex[:],
                                start=(jb == 0), stop=(jb == last))
                        # normalize and accumulate into yT
                        with tc.tile_pool(name=f"nrm{l}{hi}{ih}", bufs=2) as nrm_p:
                            dn = nrm_p.tile([32, 512], f32, name="dn", tag="dn")
                            nc.scalar.copy(dn[:], pu[YONE:YA, :])
                            rb = nrm_p.tile([32, 512], f32, name="rb", tag="rb")
                            nc.vector.reciprocal_approx_fast(rb[:], dn[:])
                            u2f = (None if hi == 0 else
                                   nrm_p.tile([YW, 512], fp8, name="u2", tag="u2"))
                            for c0, cw in ((0, 32), (32, 32), (64, 16)):
                                if hi == 0:
                                    nc.vector.scalar_tensor_tensor(
                                        yT[c0:c0 + cw, lo:hi2],
                                        pu[c0:c0 + cw, :], YS, rb[0:cw, :],
                                        AluOpType.mult, AluOpType.mult)
                                else:
                                    nc.vector.scalar_tensor_tensor(
                                        u2f[c0:c0 + cw, :],
                                        pu[c0:c0 + cw, :], YS, rb[0:cw, :],
                                        AluOpType.mult, AluOpType.mult)
                                    nc.vector.tensor_add(
                                        yT[c0:c0 + cw, lo:hi2],
                                        yT[c0:c0 + cw, lo:hi2],
                                        u2f[c0:c0 + cw, :])

                def mlp_half(l, ih, lnp, wo, w1, w2, y_out, m_in, m_out,
                             land=None):
                    """Wo + LN2 + MLP + m-AR for token half ih (after y-AR)."""
                    lo, hi2 = ih * 512, ih * 512 + 512
                    yb8 = ya_p.tile([YW, 512], fp8, name=f"yb8{l}{ih}", tag="yb8")
                    nc.sync.dma_start(yb8[:], y_out[:])
                    ybb = ya_p.tile([YW, 512], bf16, name=f"ybb{l}{ih}", tag="ybb")
                    nc.scalar.mul(ybb[:], yb8[:], 1.0 / YS)
                    for k in range(NK):
                        po = psK.tile([P, 512], f32, name="po", tag="p")
                        nc.tensor.matmul(po[:], wo[:, ts(k, P)], ybb[:],
                                         start=True, stop=True)
                        nc.vector.tensor_add(
                            xT[:, k, lo:hi2], xT[:, k, lo:hi2], po[:])
                    layernorm_half(2, 3, xn2T, lnp, f"l{l}n2h{ih}", ih)
                    if land is not None:
                        mlp_land(*land)
                    gl = gel_p.tile([P, NUB, 512], bf16, name=f"gl{l}{ih}",
                                    tag="gl")
                    for ub in range(NUB):
                        pm = psK.tile([P, 512], f32, name="pm", tag="p")
                        for kg in range(NK // 2):
                            nc.tensor.matmul(
                                pm[:], w1[:, 2 * kg:2 * kg + 2, ts(ub, P)],
                                xn2T[:, 2 * kg:2 * kg + 2, lo:hi2],
                                start=(kg == 0),
                                stop=(kg == NK // 2 - 1),
                                perf_mode=DR)
                        nc.scalar.activation(
                            gl[:, ub, :], pm[:],
                            AF.Gelu_apprx_tanh, scale=1.0 / PS)
                    # x += xn2/XS while W2 runs
                    for k in range(NK):
                        nc.vector.scalar_tensor_tensor(
                            xT[:, k, lo:hi2], xn2T[:, k, lo:hi2], 1.0 / XS,
                            xT[:, k, lo:hi2], AluOpType.mult, AluOpType.add)
                    for k in range(NK):
                        mc = mst_p.tile([P, 512], fp8, name="mc", tag="mc")
                        pp = psK.tile([P, 512], f32, name="pp", tag="p")
                        for ub in range(NUB):
                            nc.tensor.matmul(
                                pp[:], w2[:, ub, ts(k, P)], gl[:, ub, :],
                                start=(ub == 0), stop=(ub == NUB - 1))
                        nc.scalar.mul(mc[:], pp[:], MS)
                        nc.sync.dma_start(m_in[:, k, :], mc[:])
                    nc.gpsimd.collective_compute(
                        "AllReduce", AluOpType.add, replica_groups=RG,
                        ins=[m_in.opt()], outs=[m_out.opt()])

                def mlp_land(l, ih, m_out):
                    """x += mlp result for half ih (after its m-AR)."""
                    lo, hi2 = ih * 512, ih * 512 + 512
                    for k in range(NK):
                        mr = mst_p.tile([P, 512], fp8, name="mr", tag="mr")
                        nc.sync.dma_start(mr[:], m_out[:, k, :])
                        nc.vector.scalar_tensor_tensor(
                            xT[:, k, lo:hi2], mr[:], 1.0 / MS,
                            xT[:, k, lo:hi2], AluOpType.mult, AluOpType.add)

                pending_land = None
                for l in range(n_layers):
                    lnp = lnp_p.tile([P, 4, NK], f32, name=f"lnp{l}", tag="lnp")
                    nc.sync.dma_start(lnp[:], ln_e[l][:])
                    wo = lnw_p.tile([YW, D], bf16, name=f"wo{l}", tag="wo")
                    nc.sync.dma_start(wo[:], wo_e[l][:])
                    w1 = w12_p.tile([P, NK, DMS], fp8, name=f"w1{l}", tag="w1")
                    w2 = w12_p.tile([P, NUB, D], bf16, name=f"w2{l}", tag="w2")
                    nc.sync.dma_start(w1[:], w1_e[l][:])
                    nc.sync.dma_start(w2[:], w2_e[l][:])
                    wms, wvs, tTs, vhs = [], [], [], []
                    for hi in range(2):
                        wm = wm_p.tile([P, NK, D], fp8, name=f"wm{l}{hi}", tag="w")
                        nc.sync.dma_start(wm[:], wm_e[l][hi])
                        wv = vv_p.tile([P, NK, YA], fp8, name=f"wv{l}{hi}",
                                       tag="wv")
                        nc.sync.dma_start(wv[:], wv_e[l][hi])
                        wms.append(wm)
                        wvs.append(wv)
                        tTs.append(qk_p.tile([P, NK, LMAX], fp8,
                                             name=f"tT{l}{hi}", tag="qk"))
                        vhs.append(vv_p.tile([P, NJB, YA], bf16,
                                             name=f"vh{l}{hi}", tag="vh"))
                    yT = ya_p.tile([YW, LMAX], fp8, name=f"yT{l}", tag="yT")
                    y_in = [dram.tile([YW, 512], fp8, name=f"yin{l}{ih}",
                                      tag=f"yin{ih}", bufs=2) for ih in range(2)]
                    y_out = [dram.tile([YW, 512], fp8, name=f"yout{l}{ih}",
                                       tag=f"yout{ih}", addr_space="Shared",
                                       bufs=2) for ih in range(2)]
                    m_in = [dram.tile([P, NK, 512], fp8, name=f"min{l}{ih}",
                                      tag=f"min{ih}", bufs=2) for ih in range(2)]
                    m_out = [dram.tile([P, NK, 512], fp8, name=f"mout{l}{ih}",
                                       tag=f"mout{ih}", addr_space="Shared",
                                       bufs=2) for ih in range(2)]

                    layernorm_half(0, 1, xnT, lnp, f"l{l}n1h0", 0)
                    attn_half(l, 0, 0, wms[0], wvs[0], tTs[0], vhs[0], yT)
                    if pending_land is not None:
                        mlp_land(*pending_land)
                        pending_land = None
                    layernorm_half(0, 1, xnT, lnp, f"l{l}n1h1", 1)
                    attn_half(l, 1, 0, wms[1], wvs[1], tTs[1], vhs[1], yT)
                    nc.sync.dma_start(y_in[0][:], yT[:, 0:512])
                    nc.gpsimd.collective_compute(
                        "AllReduce", AluOpType.add, replica_groups=RG,
                        ins=[y_in[0].opt()], outs=[y_out[0].opt()])
                    for hi in range(2):
                        attn_half(l, hi, 1, wms[hi], wvs[hi],
                                  tTs[hi], vhs[hi], yT)
                    nc.sync.dma_start(y_in[1][:], yT[:, 512:1024])
                    nc.gpsimd.collective_compute(
                        "AllReduce", AluOpType.add, replica_groups=RG,
                        ins=[y_in[1].opt()], outs=[y_out[1].opt()])
                    if l == n_layers - 1:
                        for kg in range(NK // 2):
                            nc.sync.dma_start(wuf[:, 2 * kg:2 * kg + 2, :],
                                              wu_e[:, 2 * kg:2 * kg + 2, :])
                    mlp_half(l, 0, lnp, wo, w1, w2, y_out[0],
                             m_in[0], m_out[0])
                    mlp_half(l, 1, lnp, wo, w1, w2, y_out[1],
                             m_in[1], m_out[1], land=(l, 0, m_out[0]))
                    pending_land = (l, 1, m_out[1])
                    if taps:
                        mlp_land(*pending_land)
                        pending_land = None
                        nc.sync.dma_start(taps_e[f"dbg_x{l}"][:], xT[:])
                        yta = ya_p.tile([YW, LMAX], fp8, name=f"yta{l}",
                                        tag="yta")
                        nc.sync.dma_start(yta[:, 0:512], y_out[0][:])
                        nc.sync.dma_start(yta[:, 512:1024], y_out[1][:])
                        nc.sync.dma_start(taps_e[f"dbg_y{l}"][:], yta[:])

                # ------- final LN + unembed exp (host normalizes), -------
                # interleaved with the last layer's second m-AR landing
                unembed_half(0)
                if pending_land is not None:
                    mlp_land(*pending_land)
                unembed_half(1)
                nc.sync.dma_start(rs_e[:], rs[:])

    nc.compile()
    return nc


def shard_inputs(inputs, n_layers=N_LAYERS_BUILD):
    import ml_dtypes
    bf = ml_dtypes.bfloat16
    f8 = ml_dtypes.float8_e4m3

    x_ids = np.asarray(inputs["x_ids"]).astype(np.int64)
    we = np.asarray(inputs["word_emb"], np.float32)
    pe = np.asarray(inputs["pos_emb"], np.float32)
    x0t = np.ascontiguousarray((we[x_ids] + pe).T)  # (D, LMAX) f32
    # tile layout [p, k, i]: feature e = k*128 + p
    x0r = np.ascontiguousarray(
        x0t.reshape(NK, P, LMAX).transpose(1, 0, 2)).astype(bf)

    Wq = np.asarray(inputs["Wq"], np.float32)
    Wk = np.asarray(inputs["Wk"], np.float32)
    Wv = np.asarray(inputs["Wv"], np.float32)
    Wo = np.asarray(inputs["Wo"], np.float32)
    W1 = np.asarray(inputs["W1"], np.float32)
    W2 = np.asarray(inputs["W2"], np.float32)
    g1, b1 = np.asarray(inputs["g1"], np.float32), np.asarray(inputs["b1"], np.float32)
    g2, b2 = np.asarray(inputs["g2"], np.float32), np.asarray(inputs["b2"], np.float32)
    gf, bfv = np.asarray(inputs["gf"], np.float32), np.asarray(inputs["bf"], np.float32)
    Wu = np.asarray(inputs["Wu"], np.float32)

    tri = np.triu(np.ones((P, P), np.float32)).astype(bf)  # valid j'<=i'

    def feat_major(a):
        # (D, cols) -> (P, NK, cols) with feature e = k*128 + p
        return np.ascontiguousarray(
            a.reshape(NK, P, -1).transpose(1, 0, 2))

    # M = Wq Wk^T per (layer, head): [l, h, d, f]
    M_all = np.matmul(Wq[:n_layers], Wk[:n_layers].transpose(0, 1, 3, 2))

    in_maps = []
    for c in range(NCORES):
        m = {"x0": x0r, "trimask": tri,
             "lnf": np.ascontiguousarray(
                 (np.stack([gf * D, bfv]) * XS).astype(np.float32)
                 .reshape(2, NK, P).transpose(2, 0, 1)),
             "wu": (feat_major(Wu[:, c * VS:(c + 1) * VS]) * WS).astype(f8)}
        for l in range(n_layers):
            h0 = 2 * c
            m[f"wm{l}"] = np.stack([
                (feat_major(M_all[l, h0 + hi]) * MQS).astype(f8)
                for hi in range(2)])
            wv_eff = np.zeros((2, D, YA), np.float32)
            for hi in range(2):
                h = h0 + hi
                if h < 15:
                    wv_eff[hi, :, h] = Wv[l, h, :, 0]
                else:
                    wv_eff[hi, :, 15:15 + DV] = Wv[l, h]
                # cols 79..95 stay zero; col 96 becomes the ones column
                # (set on-chip after the matmul)
            m[f"wv{l}"] = np.stack([
                (feat_major(wv_eff[hi]) * WS).astype(f8) for hi in range(2)])
            wo80 = np.zeros((YW, D), np.float32)
            wo80[:79] = Wo[l][:79]
            m[f"wo{l}"] = wo80.astype(bf)
            m[f"w1{l}"] = (feat_major(
                W1[l][:, c * DMS:(c + 1) * DMS]) * WS).astype(f8)
            # w2 layout [p, u, d]: dm row = u*128 + p within this core's shard
            m[f"w2{l}"] = np.ascontiguousarray(
                W2[l][c * DMS:(c + 1) * DMS].reshape(NUB, P, D)
                .transpose(1, 0, 2)).astype(bf)
            m[f"ln{l}"] = np.ascontiguousarray(
                (np.stack([g1[l] * D, b1[l], g2[l] * D, b2[l]]) * XS)
                .astype(np.float32).reshape(4, NK, P).transpose(2, 0, 1))
        in_maps.append(m)
    return in_maps


_GRAPH_CACHE = {}


def _ensure_ntff_hook():
    """The agent image's antenv lacks axon_hooks; recreate it so
    run_bass_kernel_spmd(trace=True) can capture NTFF profiles."""
    import types
    try:
        import antenv.axon_hooks  # noqa: F401
        return
    except ImportError:
        pass
    import importlib.util
    import antenv
    spec = importlib.util.spec_from_file_location(
        "_trn_boot_for_hook", "/root/.axon_site/trn_agent_boot/trn_boot.py")
    tb = importlib.util.module_from_spec(spec)
    spec.loader.exec_module(tb)
    mod = types.ModuleType("antenv.axon_hooks")
    hook_box = [tb._ntff_profile_via_ctypes("/opt/axon/libaxon_pjrt.so")]
    mod.set_axon_ntff_profile_hook = lambda h: hook_box.__setitem__(0, h)
    mod.get_axon_ntff_profile_hook = lambda: hook_box[0]
    sys.modules["antenv.axon_hooks"] = mod
    antenv.axon_hooks = mod


def run(inputs, trace=False, n_layers=N_LAYERS_BUILD):
    from concourse.bass_utils import run_bass_kernel_spmd
    if trace:
        _ensure_ntff_hook()
    key = (n_layers, DEBUG_TAPS)
    if key not in _GRAPH_CACHE:
        _GRAPH_CACHE[key] = build_graph(n_layers)
    nc = _GRAPH_CACHE[key]
    in_maps = shard_inputs(inputs, n_layers)
    res = run_bass_kernel_spmd(nc, in_maps, list(range(NCORES)), trace=trace)
    expv = np.concatenate(
        [np.asarray(res.results[c]["out"], np.float32) for c in range(NCORES)],
        axis=1)                                        # (LMAX, V)
    # denominators: rs[p, ib] is the row sum of token ib*128+p on each core
    denom = np.zeros(LMAX, np.float64)
    for c in range(NCORES):
        rs = np.asarray(res.results[c]["rs"], np.float64)  # (P, NJB)
        denom += rs.T.reshape(LMAX)
    out = (expv / denom[:, None]).astype(np.float32)
    return out, res


def kernel(**inputs):
    out, _ = run(inputs)
    return out
